# revision 21
# baseline (speedup 1.0000x reference)
import os
import sys

import numpy as np

sys.path.insert(0, "/opt/trn_rl_repo")

import ml_dtypes

BETA = 0.9375
THR = 1.0
N_CORES = 8
B, NIN, NH, NOUT = 2048, 784, 128, 10
T = 100
B_CORE = B // N_CORES  # 256
NPAIR = T // 2  # 50 column tiles, each = 2 timesteps x 256 batch
NTILE = 512  # moving free dim per matmul tile (= 1 PSUM bank of fp32)
NFULL = 6  # full 128-row K chunks (768 of 784 input rows)
KREM = NIN - NFULL * 128  # 16 remainder rows
SCALE = float(2.0**11)  # hi/lo fp16 split scale; exact power of 2

_CACHE = {}

last_exec_time_ns = None


def _build_bass():
    import concourse.bass as bass
    import concourse.mybir as mybir
    import concourse.tile as tile
    from concourse import bacc
    from concourse.bass import ts

    dt = mybir.dt
    Alu = mybir.AluOpType

    nc = bacc.Bacc("TRN2", target_bir_lowering=False, debug=False)

    spk_a = nc.dram_tensor(
        "spk_a", [NPAIR // 2, 128, 2 * NFULL * NTILE], dt.float8e4, kind="ExternalInput"
    )
    spk_b = nc.dram_tensor(
        "spk_b", [2 * KREM, NPAIR * NTILE], dt.float8e4, kind="ExternalInput"
    )
    wm_d = nc.dram_tensor("wmain", [128, 2 * NFULL * NH], dt.float16, kind="ExternalInput")
    wr_d = nc.dram_tensor("wrem", [2 * KREM, NH], dt.float16, kind="ExternalInput")
    iv_d = nc.dram_tensor("init_vec", [NH, 2], dt.float32, kind="ExternalInput")
    spk_out = nc.dram_tensor(
        "spk_out", [NPAIR // 2, NH, 2 * NTILE], dt.float8e4, kind="ExternalOutput"
    )

    with tile.TileContext(nc) as tc:
        with (
            tc.tile_pool(name="wpool", bufs=1) as wpool,
            tc.tile_pool(name="state", bufs=1) as state,
            tc.tile_pool(name="spool", bufs=3) as spool,
            tc.tile_pool(name="stage", bufs=4) as stage,
            tc.tile_pool(name="tpool", bufs=3) as tpool,
            tc.psum_pool(name="ppool", bufs=4) as ppool,
        ):
            wm_sb = wpool.tile([128, 2 * NFULL * NH], dt.float16)
            wr_sb = wpool.tile([2 * KREM, NH], dt.float16)
            sb_b = wpool.tile([2 * KREM, NPAIR * NTILE], dt.float8e4)
            iv_sb = state.tile([NH, 2], dt.float32)
            # tiny DMA first: warms the scalar queue before the big weight xfer
            nc.scalar.dma_start(iv_sb[:], iv_d[:])
            nc.scalar.dma_start(wm_sb[:], wm_d[:])
            nc.scalar.dma_start(wr_sb[:], wr_d[:])
            nc.scalar.dma_start(sb_b[:, 0 : 4 * NTILE], spk_b[:, 0 : 4 * NTILE])
            nc.scalar.dma_start(sb_b[:, 4 * NTILE :], spk_b[:, 4 * NTILE :])

            mem_a = state.tile([NH, B_CORE], dt.float32)
            mem_b = state.tile([NH, B_CORE], dt.float32)
            zero_spk = state.tile([NH, B_CORE], dt.float8e4)
            nc.vector.memset(zero_spk[:], 0.0)
            thr_sb = iv_sb[:, 0:1]
            bc0_sb = iv_sb[:, 1:2]

            mems = [mem_a, mem_b]
            prev_spk = zero_spk[:]

            for n in range(NPAIR):
                if n % 2 == 0:
                    s_sb = spool.tile([128, 2 * NFULL * NTILE], dt.float8e4)
                    if n == 0:
                        h = NFULL * NTILE
                        nc.sync.dma_start(s_sb[:, 0:h], spk_a[0, :, 0:h])
                        nc.sync.dma_start(s_sb[:, h:], spk_a[0, :, h:])
                    else:
                        nc.sync.dma_start(s_sb[:], spk_a[n // 2])

                pb = ppool.tile([NH, NTILE], dt.float32)
                for c in range(NFULL):
                    nc.tensor.matmul(
                        pb[:],
                        wm_sb[:, ts(2 * c, NH)],
                        s_sb[:, ts((n % 2) * NFULL + c, NTILE)],
                        start=(c == 0),
                        stop=False,
                    )
                    nc.tensor.matmul(
                        pb[:],
                        wm_sb[:, ts(2 * c + 1, NH)],
                        s_sb[:, ts((n % 2) * NFULL + c, NTILE)],
                        start=False,
                        stop=False,
                    )
                nc.tensor.matmul(
                    pb[:],
                    wr_sb[:],
                    sb_b[:, ts(n, NTILE)],
                    start=False,
                    stop=True,
                )

                if n % 2 == 0:
                    st = stage.tile([NH, 2 * NTILE], dt.float8e4)
                for half in range(2):
                    t_idx = 2 * n + half
                    cur = pb[:, ts(half, B_CORE)]
                    m_src = mems[t_idx % 2]
                    m_dst = mems[(t_idx + 1) % 2]
                    t1 = tpool.tile([NH, B_CORE], dt.float32)
                    # t1 = cur * 2^-11 - prev_spk
                    nc.vector.scalar_tensor_tensor(
                        t1[:], cur, 1.0 / SCALE, prev_spk, Alu.mult, Alu.subtract
                    )
                    # mem' = mem * beta + t1  (step 0: mem_init is a per-row const)
                    if t_idx == 0:
                        nc.vector.tensor_scalar(
                            m_dst[:], t1[:], bc0_sb, None, Alu.add
                        )
                    else:
                        nc.vector.scalar_tensor_tensor(
                            m_dst[:], m_src[:], BETA, t1[:], Alu.mult, Alu.add
                        )
                    # spk = (mem' > thr) in {0,1}, fp8
                    spk_ap = st[:, ts(2 * (n % 2) + half, B_CORE)]
                    nc.vector.tensor_scalar(
                        spk_ap, m_dst[:], thr_sb[:], None, Alu.is_gt
                    )
                    prev_spk = spk_ap

                if n % 2 == 1:
                    nc.scalar.dma_start(spk_out[n // 2], st[:])

    nc.compile()
    return nc


def _gen_spikes(data_it):
    import jax

    cpu = jax.devices("cpu")[0]
    with jax.default_device(cpu):
        key = jax.random.key(42)
        spikes = jax.random.bernoulli(
            key, jax.numpy.asarray(data_it), (T,) + data_it.shape
        )
        return np.asarray(spikes)  # bool [T, B, NIN]


def _setup_trace_hooks():
    try:
        import antenv.axon_hooks  # noqa: F401

        return
    except ImportError:
        pass
    try:
        import types

        from trn_agent_boot.trn_boot import _ntff_profile_via_ctypes

        hook = _ntff_profile_via_ctypes("/opt/axon/libaxon_pjrt.so")
        if hook is None:
            return
        m = types.ModuleType("antenv.axon_hooks")
        m.get_axon_ntff_profile_hook = lambda: hook
        m.set_axon_ntff_profile_hook = lambda h: None
        sys.modules["antenv.axon_hooks"] = m
        from concourse import bass_utils as bu

        bu.upload_artifacts = lambda tmpdir: "local://skipped"
    except Exception as e:  # pragma: no cover
        print(f"trace hook setup failed: {e}", file=sys.stderr)


def kernel(data_it, w1, b1, w2, b2, num_steps):
    global last_exec_time_ns
    assert int(num_steps) == T

    from concourse import bass_utils

    if "nc" not in _CACHE:
        _CACHE["nc"] = _build_bass()
    nc = _CACHE["nc"]

    data_it = np.asarray(data_it, np.float32)
    w1 = np.asarray(w1, np.float32)
    b1 = np.asarray(b1, np.float32)
    w2 = np.asarray(w2, np.float32)
    b2 = np.asarray(b2, np.float32)

    spikes = _gen_spikes(data_it)  # bool [T, B, NIN]

    # weights: w1T [784, 128] -> hi/lo fp16 split, both pre-scaled by 2^11
    w1t = np.ascontiguousarray(w1.T)  # [784, 128]
    hi = w1t.astype(np.float16)
    lo32 = (w1t - hi.astype(np.float32)) * SCALE
    wh = (hi.astype(np.float32) * SCALE).astype(np.float16)
    wl = lo32.astype(np.float16)
    # full chunks: interleave [hi0, lo0, hi1, lo1, ...] along free dim
    wmain = np.stack(
        [
            wh[: NFULL * 128].reshape(NFULL, 128, NH),
            wl[: NFULL * 128].reshape(NFULL, 128, NH),
        ],
        axis=1,
    ).reshape(2 * NFULL, 128, NH)
    wmain = np.ascontiguousarray(
        wmain.transpose(1, 0, 2).reshape(128, 2 * NFULL * NH)
    )
    wrem = np.ascontiguousarray(
        np.concatenate([wh[NFULL * 128 :], wl[NFULL * 128 :]], axis=0)
    )  # [32, 128]

    c_shift = (b1 / (1.0 - BETA)).astype(np.float32)  # [128]
    init_vec = np.ascontiguousarray(
        np.stack([THR - c_shift, -BETA * c_shift], axis=1), np.float32
    )  # [128, 2]: col0 = threshold, col1 = beta * mem_init

    f8 = ml_dtypes.float8_e4m3
    in_maps = []
    for i in range(N_CORES):
        s = spikes[:, i * B_CORE : (i + 1) * B_CORE, :]  # [100, 256, 784] bool
        s = s.reshape(NPAIR, 2, B_CORE, NIN).astype(np.uint8)
        sa = s[..., : NFULL * 128].transpose(0, 3, 1, 2)  # [50, 768, 2, 256]
        sa = (
            sa.reshape(NPAIR, NFULL, 128, NTILE)
            .transpose(0, 2, 1, 3)
            .reshape(NPAIR // 2, 2, 128, NFULL * NTILE)
            .transpose(0, 2, 1, 3)
            .reshape(NPAIR // 2, 128, 2 * NFULL * NTILE)
        )
        sa = np.ascontiguousarray(sa).astype(f8)
        sb = s[..., NFULL * 128 :].transpose(0, 3, 1, 2).reshape(NPAIR, KREM, NTILE)
        sb = np.concatenate([sb, sb], axis=1)  # [50, 32, 512]
        sb = np.ascontiguousarray(
            sb.transpose(1, 0, 2).reshape(2 * KREM, NPAIR * NTILE)
        ).astype(f8)
        in_maps.append(
            {
                "spk_a": sa,
                "spk_b": sb,
                "wmain": wmain,
                "wrem": wrem,
                "init_vec": init_vec,
            }
        )

    trace = bool(os.environ.get("BASS_TRACE"))
    kw = {}
    if trace:
        _setup_trace_hooks()
        tdir = os.environ.get("BASS_TRACE_DIR")
        if tdir:
            os.makedirs(tdir, exist_ok=True)
            kw["tmpdir"] = tdir
    res = bass_utils.run_bass_kernel_spmd(
        nc, in_maps, list(range(N_CORES)), trace=trace, **kw
    )
    last_exec_time_ns = res.exec_time_ns

    # gather spk1: per core [25, 128, 1024] fp8; free = (pair k, half, batch)
    spk1_cores = []
    for i in range(N_CORES):
        o = np.asarray(res.results[i]["spk_out"]).astype(np.float32)
        o = o.reshape(NPAIR // 2, NH, 4, B_CORE).transpose(0, 2, 3, 1)
        spk1_cores.append(o.reshape(T, B_CORE, NH))
    spk1 = np.concatenate(spk1_cores, axis=1)  # [100, 2048, 128]

    # layer 2 on host, f32 like the reference
    cur2 = spk1.reshape(-1, NH) @ w2.T.astype(np.float32)
    cur2 = (cur2 + b2).reshape(T, B, NOUT).astype(np.float32)
    mem2 = np.zeros((B, NOUT), np.float32)
    spk2 = np.zeros((B, NOUT), np.float32)
    spk2_rec = np.empty((T, B, NOUT), np.float32)
    mem2_rec = np.empty((T, B, NOUT), np.float32)
    for t in range(T):
        mem2 = BETA * mem2 + cur2[t] - spk2 * THR
        spk2 = (mem2 > THR).astype(np.float32)
        spk2_rec[t] = spk2
        mem2_rec[t] = mem2
    return spk2_rec, mem2_rec


# revision 24
# speedup vs baseline: 1.0078x; 1.0078x over previous
import os
import sys

import numpy as np

sys.path.insert(0, "/opt/trn_rl_repo")

import ml_dtypes

BETA = 0.9375
THR = 1.0
N_CORES = 8
B, NIN, NH, NOUT = 2048, 784, 128, 10
T = 100
B_CORE = B // N_CORES  # 256
NPAIR = T // 2  # 50 column tiles, each = 2 timesteps x 256 batch
NTILE = 512  # moving free dim per matmul tile (= 1 PSUM bank of fp32)
NFULL = 6  # full 128-row K chunks (768 of 784 input rows)
KREM = NIN - NFULL * 128  # 16 remainder rows
SCALE = float(2.0**11)  # hi/lo fp16 split scale; exact power of 2

_CACHE = {}

last_exec_time_ns = None


def _build_bass():
    import concourse.bass as bass
    import concourse.mybir as mybir
    import concourse.tile as tile
    from concourse import bacc
    from concourse.bass import ts

    dt = mybir.dt
    Alu = mybir.AluOpType

    nc = bacc.Bacc("TRN2", target_bir_lowering=False, debug=False)

    spk_a = nc.dram_tensor(
        "spk_a", [NPAIR // 2, 128, 2 * NFULL * NTILE], dt.float8e4, kind="ExternalInput"
    )
    spk_b = nc.dram_tensor(
        "spk_b", [2 * KREM, NPAIR * NTILE], dt.float8e4, kind="ExternalInput"
    )
    wm_d = nc.dram_tensor("wmain", [128, 2 * NFULL * NH], dt.float16, kind="ExternalInput")
    wr_d = nc.dram_tensor("wrem", [2 * KREM, NH], dt.float16, kind="ExternalInput")
    iv_d = nc.dram_tensor("init_vec", [NH, 2], dt.float32, kind="ExternalInput")
    warm_d = nc.dram_tensor("warm", [1, 64], dt.float32, kind="ExternalInput")
    spk_out = nc.dram_tensor(
        "spk_out", [NPAIR // 2, NH, 2 * NTILE], dt.float8e4, kind="ExternalOutput"
    )

    with tile.TileContext(nc) as tc:
        with (
            tc.tile_pool(name="wpool", bufs=1) as wpool,
            tc.tile_pool(name="state", bufs=1) as state,
            tc.tile_pool(name="spool", bufs=3) as spool,
            tc.tile_pool(name="stage", bufs=4) as stage,
            tc.tile_pool(name="tpool", bufs=3) as tpool,
            tc.psum_pool(name="ppool", bufs=4) as ppool,
        ):
            wm_sb = wpool.tile([128, 2 * NFULL * NH], dt.float16)
            wr_sb = wpool.tile([2 * KREM, NH], dt.float16)
            sb_b = wpool.tile([2 * KREM, NPAIR * NTILE], dt.float8e4)
            iv_sb = state.tile([NH, 2], dt.float32)
            warm_sb = state.tile([1, 64], dt.float32)
            # single-packet DMA first: warms the scalar queue before the weights
            nc.scalar.dma_start(warm_sb[:], warm_d[:])
            nc.scalar.dma_start(wm_sb[:], wm_d[:])
            nc.scalar.dma_start(iv_sb[:], iv_d[:])
            nc.scalar.dma_start(wr_sb[:], wr_d[:])
            nc.scalar.dma_start(sb_b[:, 0 : 4 * NTILE], spk_b[:, 0 : 4 * NTILE])
            nc.scalar.dma_start(sb_b[:, 4 * NTILE :], spk_b[:, 4 * NTILE :])

            mem_a = state.tile([NH, B_CORE], dt.float32)
            mem_b = state.tile([NH, B_CORE], dt.float32)
            zero_spk = state.tile([NH, B_CORE], dt.float8e4)
            nc.vector.memset(zero_spk[:], 0.0)
            thr_sb = iv_sb[:, 0:1]
            bc0_sb = iv_sb[:, 1:2]

            mems = [mem_a, mem_b]
            prev_spk = zero_spk[:]

            for n in range(NPAIR):
                if n % 2 == 0:
                    s_sb = spool.tile([128, 2 * NFULL * NTILE], dt.float8e4)
                    if n == 0:
                        h = NFULL * NTILE
                        nc.sync.dma_start(s_sb[:, 0:h], spk_a[0, :, 0:h])
                        nc.sync.dma_start(s_sb[:, h:], spk_a[0, :, h:])
                    else:
                        nc.sync.dma_start(s_sb[:], spk_a[n // 2])

                pb = ppool.tile([NH, NTILE], dt.float32)
                for c in range(NFULL):
                    nc.tensor.matmul(
                        pb[:],
                        wm_sb[:, ts(2 * c, NH)],
                        s_sb[:, ts((n % 2) * NFULL + c, NTILE)],
                        start=(c == 0),
                        stop=False,
                    )
                    nc.tensor.matmul(
                        pb[:],
                        wm_sb[:, ts(2 * c + 1, NH)],
                        s_sb[:, ts((n % 2) * NFULL + c, NTILE)],
                        start=False,
                        stop=False,
                    )
                nc.tensor.matmul(
                    pb[:],
                    wr_sb[:],
                    sb_b[:, ts(n, NTILE)],
                    start=False,
                    stop=True,
                )

                if n % 2 == 0:
                    st = stage.tile([NH, 2 * NTILE], dt.float8e4)
                for half in range(2):
                    t_idx = 2 * n + half
                    cur = pb[:, ts(half, B_CORE)]
                    m_src = mems[t_idx % 2]
                    m_dst = mems[(t_idx + 1) % 2]
                    t1 = tpool.tile([NH, B_CORE], dt.float32)
                    # t1 = cur * 2^-11 - prev_spk
                    nc.vector.scalar_tensor_tensor(
                        t1[:], cur, 1.0 / SCALE, prev_spk, Alu.mult, Alu.subtract
                    )
                    # mem' = mem * beta + t1  (step 0: mem_init is a per-row const)
                    if t_idx == 0:
                        nc.vector.tensor_scalar(
                            m_dst[:], t1[:], bc0_sb, None, Alu.add
                        )
                    else:
                        nc.vector.scalar_tensor_tensor(
                            m_dst[:], m_src[:], BETA, t1[:], Alu.mult, Alu.add
                        )
                    # spk = (mem' > thr) in {0,1}, fp8
                    spk_ap = st[:, ts(2 * (n % 2) + half, B_CORE)]
                    nc.vector.tensor_scalar(
                        spk_ap, m_dst[:], thr_sb[:], None, Alu.is_gt
                    )
                    prev_spk = spk_ap

                if n % 2 == 1:
                    nc.scalar.dma_start(spk_out[n // 2], st[:])

    nc.compile()
    return nc


def _gen_spikes(data_it):
    import jax

    cpu = jax.devices("cpu")[0]
    with jax.default_device(cpu):
        key = jax.random.key(42)
        spikes = jax.random.bernoulli(
            key, jax.numpy.asarray(data_it), (T,) + data_it.shape
        )
        return np.asarray(spikes)  # bool [T, B, NIN]


def _setup_trace_hooks():
    try:
        import antenv.axon_hooks  # noqa: F401

        return
    except ImportError:
        pass
    try:
        import types

        from trn_agent_boot.trn_boot import _ntff_profile_via_ctypes

        hook = _ntff_profile_via_ctypes("/opt/axon/libaxon_pjrt.so")
        if hook is None:
            return
        m = types.ModuleType("antenv.axon_hooks")
        m.get_axon_ntff_profile_hook = lambda: hook
        m.set_axon_ntff_profile_hook = lambda h: None
        sys.modules["antenv.axon_hooks"] = m
        from concourse import bass_utils as bu

        bu.upload_artifacts = lambda tmpdir: "local://skipped"
    except Exception as e:  # pragma: no cover
        print(f"trace hook setup failed: {e}", file=sys.stderr)


def kernel(data_it, w1, b1, w2, b2, num_steps):
    global last_exec_time_ns
    assert int(num_steps) == T

    from concourse import bass_utils

    if "nc" not in _CACHE:
        _CACHE["nc"] = _build_bass()
    nc = _CACHE["nc"]

    data_it = np.asarray(data_it, np.float32)
    w1 = np.asarray(w1, np.float32)
    b1 = np.asarray(b1, np.float32)
    w2 = np.asarray(w2, np.float32)
    b2 = np.asarray(b2, np.float32)

    spikes = _gen_spikes(data_it)  # bool [T, B, NIN]

    # weights: w1T [784, 128] -> hi/lo fp16 split, both pre-scaled by 2^11
    w1t = np.ascontiguousarray(w1.T)  # [784, 128]
    hi = w1t.astype(np.float16)
    lo32 = (w1t - hi.astype(np.float32)) * SCALE
    wh = (hi.astype(np.float32) * SCALE).astype(np.float16)
    wl = lo32.astype(np.float16)
    # full chunks: interleave [hi0, lo0, hi1, lo1, ...] along free dim
    wmain = np.stack(
        [
            wh[: NFULL * 128].reshape(NFULL, 128, NH),
            wl[: NFULL * 128].reshape(NFULL, 128, NH),
        ],
        axis=1,
    ).reshape(2 * NFULL, 128, NH)
    wmain = np.ascontiguousarray(
        wmain.transpose(1, 0, 2).reshape(128, 2 * NFULL * NH)
    )
    wrem = np.ascontiguousarray(
        np.concatenate([wh[NFULL * 128 :], wl[NFULL * 128 :]], axis=0)
    )  # [32, 128]

    c_shift = (b1 / (1.0 - BETA)).astype(np.float32)  # [128]
    init_vec = np.ascontiguousarray(
        np.stack([THR - c_shift, -BETA * c_shift], axis=1), np.float32
    )  # [128, 2]: col0 = threshold, col1 = beta * mem_init

    f8 = ml_dtypes.float8_e4m3
    in_maps = []
    for i in range(N_CORES):
        s = spikes[:, i * B_CORE : (i + 1) * B_CORE, :]  # [100, 256, 784] bool
        s = s.reshape(NPAIR, 2, B_CORE, NIN).astype(np.uint8)
        sa = s[..., : NFULL * 128].transpose(0, 3, 1, 2)  # [50, 768, 2, 256]
        sa = (
            sa.reshape(NPAIR, NFULL, 128, NTILE)
            .transpose(0, 2, 1, 3)
            .reshape(NPAIR // 2, 2, 128, NFULL * NTILE)
            .transpose(0, 2, 1, 3)
            .reshape(NPAIR // 2, 128, 2 * NFULL * NTILE)
        )
        sa = np.ascontiguousarray(sa).astype(f8)
        sb = s[..., NFULL * 128 :].transpose(0, 3, 1, 2).reshape(NPAIR, KREM, NTILE)
        sb = np.concatenate([sb, sb], axis=1)  # [50, 32, 512]
        sb = np.ascontiguousarray(
            sb.transpose(1, 0, 2).reshape(2 * KREM, NPAIR * NTILE)
        ).astype(f8)
        in_maps.append(
            {
                "spk_a": sa,
                "spk_b": sb,
                "wmain": wmain,
                "wrem": wrem,
                "init_vec": init_vec,
                "warm": np.zeros((1, 64), np.float32),
            }
        )

    trace = bool(os.environ.get("BASS_TRACE"))
    kw = {}
    if trace:
        _setup_trace_hooks()
        tdir = os.environ.get("BASS_TRACE_DIR")
        if tdir:
            os.makedirs(tdir, exist_ok=True)
            kw["tmpdir"] = tdir
    res = bass_utils.run_bass_kernel_spmd(
        nc, in_maps, list(range(N_CORES)), trace=trace, **kw
    )
    last_exec_time_ns = res.exec_time_ns

    # gather spk1: per core [25, 128, 1024] fp8; free = (pair k, half, batch)
    spk1_cores = []
    for i in range(N_CORES):
        o = np.asarray(res.results[i]["spk_out"]).astype(np.float32)
        o = o.reshape(NPAIR // 2, NH, 4, B_CORE).transpose(0, 2, 3, 1)
        spk1_cores.append(o.reshape(T, B_CORE, NH))
    spk1 = np.concatenate(spk1_cores, axis=1)  # [100, 2048, 128]

    # layer 2 on host, f32 like the reference
    cur2 = spk1.reshape(-1, NH) @ w2.T.astype(np.float32)
    cur2 = (cur2 + b2).reshape(T, B, NOUT).astype(np.float32)
    mem2 = np.zeros((B, NOUT), np.float32)
    spk2 = np.zeros((B, NOUT), np.float32)
    spk2_rec = np.empty((T, B, NOUT), np.float32)
    mem2_rec = np.empty((T, B, NOUT), np.float32)
    for t in range(T):
        mem2 = BETA * mem2 + cur2[t] - spk2 * THR
        spk2 = (mem2 > THR).astype(np.float32)
        spk2_rec[t] = spk2
        mem2_rec[t] = mem2
    return spk2_rec, mem2_rec


# revision 27
# speedup vs baseline: 1.0149x; 1.0071x over previous
import os
import sys

import numpy as np

sys.path.insert(0, "/opt/trn_rl_repo")

import ml_dtypes

BETA = 0.9375
THR = 1.0
N_CORES = 8
B, NIN, NH, NOUT = 2048, 784, 128, 10
T = 100
B_CORE = B // N_CORES  # 256
NPAIR = T // 2  # 50 column tiles, each = 2 timesteps x 256 batch
NTILE = 512  # moving free dim per matmul tile (= 1 PSUM bank of fp32)
NFULL = 6  # full 128-row K chunks (768 of 784 input rows)
KREM = NIN - NFULL * 128  # 16 remainder rows
SCALE = float(2.0**11)  # hi/lo fp16 split scale; exact power of 2

_CACHE = {}

last_exec_time_ns = None


def _build_bass():
    import concourse.bass as bass
    import concourse.mybir as mybir
    import concourse.tile as tile
    from concourse import bacc
    from concourse.bass import ts

    dt = mybir.dt
    Alu = mybir.AluOpType

    nc = bacc.Bacc("TRN2", target_bir_lowering=False, debug=False)

    spk_a = nc.dram_tensor(
        "spk_a", [NPAIR // 2, 128, 2 * NFULL * NTILE], dt.float8e4, kind="ExternalInput"
    )
    spk_b = nc.dram_tensor(
        "spk_b", [2 * KREM, NPAIR * NTILE], dt.float8e4, kind="ExternalInput"
    )
    wm_d = nc.dram_tensor("wmain", [128, 2 * NFULL * NH], dt.float16, kind="ExternalInput")
    wr_d = nc.dram_tensor("wrem", [2 * KREM, NH], dt.float16, kind="ExternalInput")
    iv_d = nc.dram_tensor("init_vec", [NH, 2], dt.float32, kind="ExternalInput")
    spk_out = nc.dram_tensor(
        "spk_out", [NPAIR // 2, NH, 2 * NTILE], dt.float8e4, kind="ExternalOutput"
    )

    with tile.TileContext(nc) as tc:
        with (
            tc.tile_pool(name="wpool", bufs=1) as wpool,
            tc.tile_pool(name="state", bufs=1) as state,
            tc.tile_pool(name="spool", bufs=3) as spool,
            tc.tile_pool(name="stage", bufs=4) as stage,
            tc.tile_pool(name="tpool", bufs=3) as tpool,
            tc.psum_pool(name="ppool", bufs=4) as ppool,
        ):
            wm_sb = wpool.tile([128, 2 * NFULL * NH], dt.float16)
            wr_sb = wpool.tile([2 * KREM, NH], dt.float16)
            sb_b = wpool.tile([2 * KREM, NPAIR * NTILE], dt.float8e4)
            iv_sb = state.tile([NH, 2], dt.float32)
            # weights ride the sync queue first: it starts ~2us earlier than scalar's
            nc.sync.dma_start(wm_sb[:], wm_d[:])
            nc.scalar.dma_start(iv_sb[:], iv_d[:])
            nc.scalar.dma_start(wr_sb[:], wr_d[:])
            nc.scalar.dma_start(sb_b[:, 0 : 4 * NTILE], spk_b[:, 0 : 4 * NTILE])
            nc.scalar.dma_start(sb_b[:, 4 * NTILE :], spk_b[:, 4 * NTILE :])

            mem_a = state.tile([NH, B_CORE], dt.float32)
            mem_b = state.tile([NH, B_CORE], dt.float32)
            zero_spk = state.tile([NH, B_CORE], dt.float8e4)
            nc.vector.memset(zero_spk[:], 0.0)
            thr_sb = iv_sb[:, 0:1]
            bc0_sb = iv_sb[:, 1:2]

            mems = [mem_a, mem_b]
            prev_spk = zero_spk[:]

            for n in range(NPAIR):
                if n % 2 == 0:
                    s_sb = spool.tile([128, 2 * NFULL * NTILE], dt.float8e4)
                    if n == 0:
                        h = NFULL * NTILE
                        nc.sync.dma_start(s_sb[:, 0:h], spk_a[0, :, 0:h])
                        nc.sync.dma_start(s_sb[:, h:], spk_a[0, :, h:])
                    else:
                        nc.sync.dma_start(s_sb[:], spk_a[n // 2])

                pb = ppool.tile([NH, NTILE], dt.float32)
                for c in range(NFULL):
                    nc.tensor.matmul(
                        pb[:],
                        wm_sb[:, ts(2 * c, NH)],
                        s_sb[:, ts((n % 2) * NFULL + c, NTILE)],
                        start=(c == 0),
                        stop=False,
                    )
                    nc.tensor.matmul(
                        pb[:],
                        wm_sb[:, ts(2 * c + 1, NH)],
                        s_sb[:, ts((n % 2) * NFULL + c, NTILE)],
                        start=False,
                        stop=False,
                    )
                nc.tensor.matmul(
                    pb[:],
                    wr_sb[:],
                    sb_b[:, ts(n, NTILE)],
                    start=False,
                    stop=True,
                )

                if n % 2 == 0:
                    st = stage.tile([NH, 2 * NTILE], dt.float8e4)
                for half in range(2):
                    t_idx = 2 * n + half
                    cur = pb[:, ts(half, B_CORE)]
                    m_src = mems[t_idx % 2]
                    m_dst = mems[(t_idx + 1) % 2]
                    t1 = tpool.tile([NH, B_CORE], dt.float32)
                    # t1 = cur * 2^-11 - prev_spk
                    nc.vector.scalar_tensor_tensor(
                        t1[:], cur, 1.0 / SCALE, prev_spk, Alu.mult, Alu.subtract
                    )
                    # mem' = mem * beta + t1  (step 0: mem_init is a per-row const)
                    if t_idx == 0:
                        nc.vector.tensor_scalar(
                            m_dst[:], t1[:], bc0_sb, None, Alu.add
                        )
                    else:
                        nc.vector.scalar_tensor_tensor(
                            m_dst[:], m_src[:], BETA, t1[:], Alu.mult, Alu.add
                        )
                    # spk = (mem' > thr) in {0,1}, fp8
                    spk_ap = st[:, ts(2 * (n % 2) + half, B_CORE)]
                    nc.vector.tensor_scalar(
                        spk_ap, m_dst[:], thr_sb[:], None, Alu.is_gt
                    )
                    prev_spk = spk_ap

                if n % 2 == 1:
                    nc.scalar.dma_start(spk_out[n // 2], st[:])

    nc.compile()
    return nc


def _gen_spikes(data_it):
    import jax

    cpu = jax.devices("cpu")[0]
    with jax.default_device(cpu):
        key = jax.random.key(42)
        spikes = jax.random.bernoulli(
            key, jax.numpy.asarray(data_it), (T,) + data_it.shape
        )
        return np.asarray(spikes)  # bool [T, B, NIN]


def _setup_trace_hooks():
    try:
        import antenv.axon_hooks  # noqa: F401

        return
    except ImportError:
        pass
    try:
        import types

        from trn_agent_boot.trn_boot import _ntff_profile_via_ctypes

        hook = _ntff_profile_via_ctypes("/opt/axon/libaxon_pjrt.so")
        if hook is None:
            return
        m = types.ModuleType("antenv.axon_hooks")
        m.get_axon_ntff_profile_hook = lambda: hook
        m.set_axon_ntff_profile_hook = lambda h: None
        sys.modules["antenv.axon_hooks"] = m
        from concourse import bass_utils as bu

        bu.upload_artifacts = lambda tmpdir: "local://skipped"
    except Exception as e:  # pragma: no cover
        print(f"trace hook setup failed: {e}", file=sys.stderr)


def kernel(data_it, w1, b1, w2, b2, num_steps):
    global last_exec_time_ns
    assert int(num_steps) == T

    from concourse import bass_utils

    if "nc" not in _CACHE:
        _CACHE["nc"] = _build_bass()
    nc = _CACHE["nc"]

    data_it = np.asarray(data_it, np.float32)
    w1 = np.asarray(w1, np.float32)
    b1 = np.asarray(b1, np.float32)
    w2 = np.asarray(w2, np.float32)
    b2 = np.asarray(b2, np.float32)

    spikes = _gen_spikes(data_it)  # bool [T, B, NIN]

    # weights: w1T [784, 128] -> hi/lo fp16 split, both pre-scaled by 2^11
    w1t = np.ascontiguousarray(w1.T)  # [784, 128]
    hi = w1t.astype(np.float16)
    lo32 = (w1t - hi.astype(np.float32)) * SCALE
    wh = (hi.astype(np.float32) * SCALE).astype(np.float16)
    wl = lo32.astype(np.float16)
    # full chunks: interleave [hi0, lo0, hi1, lo1, ...] along free dim
    wmain = np.stack(
        [
            wh[: NFULL * 128].reshape(NFULL, 128, NH),
            wl[: NFULL * 128].reshape(NFULL, 128, NH),
        ],
        axis=1,
    ).reshape(2 * NFULL, 128, NH)
    wmain = np.ascontiguousarray(
        wmain.transpose(1, 0, 2).reshape(128, 2 * NFULL * NH)
    )
    wrem = np.ascontiguousarray(
        np.concatenate([wh[NFULL * 128 :], wl[NFULL * 128 :]], axis=0)
    )  # [32, 128]

    c_shift = (b1 / (1.0 - BETA)).astype(np.float32)  # [128]
    init_vec = np.ascontiguousarray(
        np.stack([THR - c_shift, -BETA * c_shift], axis=1), np.float32
    )  # [128, 2]: col0 = threshold, col1 = beta * mem_init

    f8 = ml_dtypes.float8_e4m3
    in_maps = []
    for i in range(N_CORES):
        s = spikes[:, i * B_CORE : (i + 1) * B_CORE, :]  # [100, 256, 784] bool
        s = s.reshape(NPAIR, 2, B_CORE, NIN).astype(np.uint8)
        sa = s[..., : NFULL * 128].transpose(0, 3, 1, 2)  # [50, 768, 2, 256]
        sa = (
            sa.reshape(NPAIR, NFULL, 128, NTILE)
            .transpose(0, 2, 1, 3)
            .reshape(NPAIR // 2, 2, 128, NFULL * NTILE)
            .transpose(0, 2, 1, 3)
            .reshape(NPAIR // 2, 128, 2 * NFULL * NTILE)
        )
        sa = np.ascontiguousarray(sa).astype(f8)
        sb = s[..., NFULL * 128 :].transpose(0, 3, 1, 2).reshape(NPAIR, KREM, NTILE)
        sb = np.concatenate([sb, sb], axis=1)  # [50, 32, 512]
        sb = np.ascontiguousarray(
            sb.transpose(1, 0, 2).reshape(2 * KREM, NPAIR * NTILE)
        ).astype(f8)
        in_maps.append(
            {
                "spk_a": sa,
                "spk_b": sb,
                "wmain": wmain,
                "wrem": wrem,
                "init_vec": init_vec,
            }
        )

    trace = bool(os.environ.get("BASS_TRACE"))
    kw = {}
    if trace:
        _setup_trace_hooks()
        tdir = os.environ.get("BASS_TRACE_DIR")
        if tdir:
            os.makedirs(tdir, exist_ok=True)
            kw["tmpdir"] = tdir
    res = bass_utils.run_bass_kernel_spmd(
        nc, in_maps, list(range(N_CORES)), trace=trace, **kw
    )
    last_exec_time_ns = res.exec_time_ns

    # gather spk1: per core [25, 128, 1024] fp8; free = (pair k, half, batch)
    spk1_cores = []
    for i in range(N_CORES):
        o = np.asarray(res.results[i]["spk_out"]).astype(np.float32)
        o = o.reshape(NPAIR // 2, NH, 4, B_CORE).transpose(0, 2, 3, 1)
        spk1_cores.append(o.reshape(T, B_CORE, NH))
    spk1 = np.concatenate(spk1_cores, axis=1)  # [100, 2048, 128]

    # layer 2 on host, f32 like the reference
    cur2 = spk1.reshape(-1, NH) @ w2.T.astype(np.float32)
    cur2 = (cur2 + b2).reshape(T, B, NOUT).astype(np.float32)
    mem2 = np.zeros((B, NOUT), np.float32)
    spk2 = np.zeros((B, NOUT), np.float32)
    spk2_rec = np.empty((T, B, NOUT), np.float32)
    mem2_rec = np.empty((T, B, NOUT), np.float32)
    for t in range(T):
        mem2 = BETA * mem2 + cur2[t] - spk2 * THR
        spk2 = (mem2 > THR).astype(np.float32)
        spk2_rec[t] = spk2
        mem2_rec[t] = mem2
    return spk2_rec, mem2_rec


# revision 29
# speedup vs baseline: 1.0224x; 1.0073x over previous
import os
import sys

import numpy as np

sys.path.insert(0, "/opt/trn_rl_repo")

import ml_dtypes

BETA = 0.9375
THR = 1.0
N_CORES = 8
B, NIN, NH, NOUT = 2048, 784, 128, 10
T = 100
B_CORE = B // N_CORES  # 256
NPAIR = T // 2  # 50 column tiles, each = 2 timesteps x 256 batch
NTILE = 512  # moving free dim per matmul tile (= 1 PSUM bank of fp32)
NFULL = 6  # full 128-row K chunks (768 of 784 input rows)
KREM = NIN - NFULL * 128  # 16 remainder rows
SCALE = float(2.0**11)  # hi/lo fp16 split scale; exact power of 2

_CACHE = {}

last_exec_time_ns = None


def _build_bass():
    import concourse.bass as bass
    import concourse.mybir as mybir
    import concourse.tile as tile
    from concourse import bacc
    from concourse.bass import ts

    dt = mybir.dt
    Alu = mybir.AluOpType

    nc = bacc.Bacc("TRN2", target_bir_lowering=False, debug=False)

    spk_a = nc.dram_tensor(
        "spk_a", [NPAIR // 2, 128, 2 * NFULL * NTILE], dt.float8e4, kind="ExternalInput"
    )
    spk_b = nc.dram_tensor(
        "spk_b", [2 * KREM, NPAIR * NTILE], dt.float8e4, kind="ExternalInput"
    )
    wm_d = nc.dram_tensor("wmain", [128, 2 * NFULL * NH], dt.float16, kind="ExternalInput")
    wr_d = nc.dram_tensor("wrem", [2 * KREM, NH], dt.float16, kind="ExternalInput")
    iv_d = nc.dram_tensor("init_vec", [NH, 2], dt.float32, kind="ExternalInput")
    spk_out = nc.dram_tensor(
        "spk_out", [NPAIR // 2, NH, 2 * NTILE], dt.float8e4, kind="ExternalOutput"
    )

    with tile.TileContext(nc) as tc:
        with (
            tc.tile_pool(name="wpool", bufs=1) as wpool,
            tc.tile_pool(name="state", bufs=1) as state,
            tc.tile_pool(name="spool", bufs=3) as spool,
            tc.tile_pool(name="stage", bufs=4) as stage,
            tc.tile_pool(name="tpool", bufs=3) as tpool,
            tc.psum_pool(name="ppool", bufs=4) as ppool,
        ):
            wm_sb = wpool.tile([128, 2 * NFULL * NH], dt.float16)
            wr_sb = wpool.tile([2 * KREM, NH], dt.float16)
            sb_b = wpool.tile([2 * KREM, NPAIR * NTILE], dt.float8e4)
            iv_sb = state.tile([NH, 2], dt.float32)
            # weights ride the sync queue first: it starts ~2us earlier than scalar's
            nc.sync.dma_start(wm_sb[:], wm_d[:])
            nc.scalar.dma_start(iv_sb[:], iv_d[:])
            nc.scalar.dma_start(wr_sb[:], wr_d[:])
            nc.scalar.dma_start(sb_b[:, 0 : 8 * NTILE], spk_b[:, 0 : 8 * NTILE])

            mem_a = state.tile([NH, B_CORE], dt.float32)
            mem_b = state.tile([NH, B_CORE], dt.float32)
            zero_spk = state.tile([NH, B_CORE], dt.float8e4)
            nc.vector.memset(zero_spk[:], 0.0)
            thr_sb = iv_sb[:, 0:1]
            bc0_sb = iv_sb[:, 1:2]

            mems = [mem_a, mem_b]
            prev_spk = zero_spk[:]

            for n in range(NPAIR):
                if n % 2 == 0:
                    s_sb = spool.tile([128, 2 * NFULL * NTILE], dt.float8e4)
                    if n == 0:
                        h = NFULL * NTILE
                        nc.sync.dma_start(s_sb[:, 0:h], spk_a[0, :, 0:h])
                        nc.sync.dma_start(s_sb[:, h:], spk_a[0, :, h:])
                    else:
                        nc.sync.dma_start(s_sb[:], spk_a[n // 2])

                pb = ppool.tile([NH, NTILE], dt.float32)
                for c in range(NFULL):
                    nc.tensor.matmul(
                        pb[:],
                        wm_sb[:, ts(2 * c, NH)],
                        s_sb[:, ts((n % 2) * NFULL + c, NTILE)],
                        start=(c == 0),
                        stop=False,
                    )
                    nc.tensor.matmul(
                        pb[:],
                        wm_sb[:, ts(2 * c + 1, NH)],
                        s_sb[:, ts((n % 2) * NFULL + c, NTILE)],
                        start=False,
                        stop=False,
                    )
                nc.tensor.matmul(
                    pb[:],
                    wr_sb[:],
                    sb_b[:, ts(n, NTILE)],
                    start=False,
                    stop=True,
                )

                if n % 2 == 0:
                    st = stage.tile([NH, 2 * NTILE], dt.float8e4)
                for half in range(2):
                    t_idx = 2 * n + half
                    cur = pb[:, ts(half, B_CORE)]
                    m_src = mems[t_idx % 2]
                    m_dst = mems[(t_idx + 1) % 2]
                    t1 = tpool.tile([NH, B_CORE], dt.float32)
                    # t1 = cur * 2^-11 - prev_spk
                    nc.vector.scalar_tensor_tensor(
                        t1[:], cur, 1.0 / SCALE, prev_spk, Alu.mult, Alu.subtract
                    )
                    # mem' = mem * beta + t1  (step 0: mem_init is a per-row const)
                    if t_idx == 0:
                        nc.vector.tensor_scalar(
                            m_dst[:], t1[:], bc0_sb, None, Alu.add
                        )
                    else:
                        nc.vector.scalar_tensor_tensor(
                            m_dst[:], m_src[:], BETA, t1[:], Alu.mult, Alu.add
                        )
                    # spk = (mem' > thr) in {0,1}, fp8
                    spk_ap = st[:, ts(2 * (n % 2) + half, B_CORE)]
                    nc.vector.tensor_scalar(
                        spk_ap, m_dst[:], thr_sb[:], None, Alu.is_gt
                    )
                    prev_spk = spk_ap

                if n % 2 == 1:
                    nc.scalar.dma_start(spk_out[n // 2], st[:])
                if n == 1:
                    # deferred: big remainder xfer would starve the spike queue
                    # during the startup crunch if issued up front
                    nc.scalar.dma_start(sb_b[:, 8 * NTILE :], spk_b[:, 8 * NTILE :])

    nc.compile()
    return nc


def _gen_spikes(data_it):
    import jax

    cpu = jax.devices("cpu")[0]
    with jax.default_device(cpu):
        key = jax.random.key(42)
        spikes = jax.random.bernoulli(
            key, jax.numpy.asarray(data_it), (T,) + data_it.shape
        )
        return np.asarray(spikes)  # bool [T, B, NIN]


def _setup_trace_hooks():
    try:
        import antenv.axon_hooks  # noqa: F401

        return
    except ImportError:
        pass
    try:
        import types

        from trn_agent_boot.trn_boot import _ntff_profile_via_ctypes

        hook = _ntff_profile_via_ctypes("/opt/axon/libaxon_pjrt.so")
        if hook is None:
            return
        m = types.ModuleType("antenv.axon_hooks")
        m.get_axon_ntff_profile_hook = lambda: hook
        m.set_axon_ntff_profile_hook = lambda h: None
        sys.modules["antenv.axon_hooks"] = m
        from concourse import bass_utils as bu

        bu.upload_artifacts = lambda tmpdir: "local://skipped"
    except Exception as e:  # pragma: no cover
        print(f"trace hook setup failed: {e}", file=sys.stderr)


def kernel(data_it, w1, b1, w2, b2, num_steps):
    global last_exec_time_ns
    assert int(num_steps) == T

    from concourse import bass_utils

    if "nc" not in _CACHE:
        _CACHE["nc"] = _build_bass()
    nc = _CACHE["nc"]

    data_it = np.asarray(data_it, np.float32)
    w1 = np.asarray(w1, np.float32)
    b1 = np.asarray(b1, np.float32)
    w2 = np.asarray(w2, np.float32)
    b2 = np.asarray(b2, np.float32)

    spikes = _gen_spikes(data_it)  # bool [T, B, NIN]

    # weights: w1T [784, 128] -> hi/lo fp16 split, both pre-scaled by 2^11
    w1t = np.ascontiguousarray(w1.T)  # [784, 128]
    hi = w1t.astype(np.float16)
    lo32 = (w1t - hi.astype(np.float32)) * SCALE
    wh = (hi.astype(np.float32) * SCALE).astype(np.float16)
    wl = lo32.astype(np.float16)
    # full chunks: interleave [hi0, lo0, hi1, lo1, ...] along free dim
    wmain = np.stack(
        [
            wh[: NFULL * 128].reshape(NFULL, 128, NH),
            wl[: NFULL * 128].reshape(NFULL, 128, NH),
        ],
        axis=1,
    ).reshape(2 * NFULL, 128, NH)
    wmain = np.ascontiguousarray(
        wmain.transpose(1, 0, 2).reshape(128, 2 * NFULL * NH)
    )
    wrem = np.ascontiguousarray(
        np.concatenate([wh[NFULL * 128 :], wl[NFULL * 128 :]], axis=0)
    )  # [32, 128]

    c_shift = (b1 / (1.0 - BETA)).astype(np.float32)  # [128]
    init_vec = np.ascontiguousarray(
        np.stack([THR - c_shift, -BETA * c_shift], axis=1), np.float32
    )  # [128, 2]: col0 = threshold, col1 = beta * mem_init

    f8 = ml_dtypes.float8_e4m3
    in_maps = []
    for i in range(N_CORES):
        s = spikes[:, i * B_CORE : (i + 1) * B_CORE, :]  # [100, 256, 784] bool
        s = s.reshape(NPAIR, 2, B_CORE, NIN).astype(np.uint8)
        sa = s[..., : NFULL * 128].transpose(0, 3, 1, 2)  # [50, 768, 2, 256]
        sa = (
            sa.reshape(NPAIR, NFULL, 128, NTILE)
            .transpose(0, 2, 1, 3)
            .reshape(NPAIR // 2, 2, 128, NFULL * NTILE)
            .transpose(0, 2, 1, 3)
            .reshape(NPAIR // 2, 128, 2 * NFULL * NTILE)
        )
        sa = np.ascontiguousarray(sa).astype(f8)
        sb = s[..., NFULL * 128 :].transpose(0, 3, 1, 2).reshape(NPAIR, KREM, NTILE)
        sb = np.concatenate([sb, sb], axis=1)  # [50, 32, 512]
        sb = np.ascontiguousarray(
            sb.transpose(1, 0, 2).reshape(2 * KREM, NPAIR * NTILE)
        ).astype(f8)
        in_maps.append(
            {
                "spk_a": sa,
                "spk_b": sb,
                "wmain": wmain,
                "wrem": wrem,
                "init_vec": init_vec,
            }
        )

    trace = bool(os.environ.get("BASS_TRACE"))
    kw = {}
    if trace:
        _setup_trace_hooks()
        tdir = os.environ.get("BASS_TRACE_DIR")
        if tdir:
            os.makedirs(tdir, exist_ok=True)
            kw["tmpdir"] = tdir
    res = bass_utils.run_bass_kernel_spmd(
        nc, in_maps, list(range(N_CORES)), trace=trace, **kw
    )
    last_exec_time_ns = res.exec_time_ns

    # gather spk1: per core [25, 128, 1024] fp8; free = (pair k, half, batch)
    spk1_cores = []
    for i in range(N_CORES):
        o = np.asarray(res.results[i]["spk_out"]).astype(np.float32)
        o = o.reshape(NPAIR // 2, NH, 4, B_CORE).transpose(0, 2, 3, 1)
        spk1_cores.append(o.reshape(T, B_CORE, NH))
    spk1 = np.concatenate(spk1_cores, axis=1)  # [100, 2048, 128]

    # layer 2 on host, f32 like the reference
    cur2 = spk1.reshape(-1, NH) @ w2.T.astype(np.float32)
    cur2 = (cur2 + b2).reshape(T, B, NOUT).astype(np.float32)
    mem2 = np.zeros((B, NOUT), np.float32)
    spk2 = np.zeros((B, NOUT), np.float32)
    spk2_rec = np.empty((T, B, NOUT), np.float32)
    mem2_rec = np.empty((T, B, NOUT), np.float32)
    for t in range(T):
        mem2 = BETA * mem2 + cur2[t] - spk2 * THR
        spk2 = (mem2 > THR).astype(np.float32)
        spk2_rec[t] = spk2
        mem2_rec[t] = mem2
    return spk2_rec, mem2_rec


# revision 32
# speedup vs baseline: 1.0322x; 1.0096x over previous
import os
import sys

import numpy as np

sys.path.insert(0, "/opt/trn_rl_repo")

import ml_dtypes

BETA = 0.9375
THR = 1.0
N_CORES = 8
B, NIN, NH, NOUT = 2048, 784, 128, 10
T = 100
B_CORE = B // N_CORES  # 256
NPAIR = T // 2  # 50 column tiles, each = 2 timesteps x 256 batch
NTILE = 512  # moving free dim per matmul tile (= 1 PSUM bank of fp32)
NFULL = 6  # full 128-row K chunks (768 of 784 input rows)
KREM = NIN - NFULL * 128  # 16 remainder rows
SCALE = float(2.0**11)  # hi/lo fp16 split scale; exact power of 2

_CACHE = {}

last_exec_time_ns = None


def _build_bass():
    import concourse.bass as bass
    import concourse.mybir as mybir
    import concourse.tile as tile
    from concourse import bacc
    from concourse.bass import ts

    dt = mybir.dt
    Alu = mybir.AluOpType

    nc = bacc.Bacc("TRN2", target_bir_lowering=False, debug=False)

    spk_a = nc.dram_tensor(
        "spk_a", [NPAIR // 2, 128, 2 * NFULL * NTILE], dt.float8e4, kind="ExternalInput"
    )
    spk_b = nc.dram_tensor(
        "spk_b", [2 * KREM, NPAIR * NTILE], dt.float8e4, kind="ExternalInput"
    )
    wm_d = nc.dram_tensor("wmain", [128, 2 * NFULL * NH], dt.float16, kind="ExternalInput")
    wr_d = nc.dram_tensor("wrem", [2 * KREM, NH], dt.float16, kind="ExternalInput")
    iv_d = nc.dram_tensor("init_vec", [NH, 2], dt.float32, kind="ExternalInput")
    spk_out = nc.dram_tensor(
        "spk_out", [NPAIR // 2, NH, 2 * NTILE], dt.float8e4, kind="ExternalOutput"
    )

    with tile.TileContext(nc) as tc:
        with (
            tc.tile_pool(name="wpool", bufs=1) as wpool,
            tc.tile_pool(name="state", bufs=1) as state,
            tc.tile_pool(name="spool", bufs=3) as spool,
            tc.tile_pool(name="stage", bufs=4) as stage,
            tc.tile_pool(name="tpool", bufs=3) as tpool,
            tc.psum_pool(name="ppool", bufs=4) as ppool,
        ):
            wm_sb = wpool.tile([128, 2 * NFULL * NH], dt.float16)
            wr_sb = wpool.tile([2 * KREM, NH], dt.float16)
            sb_b = wpool.tile([2 * KREM, NPAIR * NTILE], dt.float8e4)
            iv_sb = state.tile([NH, 2], dt.float32)
            # weights ride the sync queue first: it starts ~2us earlier than scalar's
            nc.sync.dma_start(wm_sb[:], wm_d[:])
            nc.scalar.dma_start(wr_sb[:], wr_d[:])
            nc.scalar.dma_start(sb_b[:, 0 : 8 * NTILE], spk_b[:, 0 : 8 * NTILE])
            # iv goes last: 128 tiny packets would delay the transfers above
            nc.scalar.dma_start(iv_sb[:], iv_d[:])

            mem_a = state.tile([NH, B_CORE], dt.float32)
            mem_b = state.tile([NH, B_CORE], dt.float32)
            zero_spk = state.tile([NH, B_CORE], dt.float8e4)
            nc.vector.memset(zero_spk[:], 0.0)
            thr_sb = iv_sb[:, 0:1]
            bc0_sb = iv_sb[:, 1:2]

            mems = [mem_a, mem_b]
            prev_spk = zero_spk[:]

            for n in range(NPAIR):
                if n % 2 == 0:
                    s_sb = spool.tile([128, 2 * NFULL * NTILE], dt.float8e4)
                    if n == 0:
                        h = NFULL * NTILE
                        nc.sync.dma_start(s_sb[:, 0 : h // 2], spk_a[0, :, 0 : h // 2])
                        nc.sync.dma_start(s_sb[:, h // 2 : h], spk_a[0, :, h // 2 : h])
                        nc.sync.dma_start(s_sb[:, h:], spk_a[0, :, h:])
                    else:
                        nc.sync.dma_start(s_sb[:], spk_a[n // 2])

                pb = ppool.tile([NH, NTILE], dt.float32)
                for c in range(NFULL):
                    nc.tensor.matmul(
                        pb[:],
                        wm_sb[:, ts(2 * c, NH)],
                        s_sb[:, ts((n % 2) * NFULL + c, NTILE)],
                        start=(c == 0),
                        stop=False,
                    )
                    nc.tensor.matmul(
                        pb[:],
                        wm_sb[:, ts(2 * c + 1, NH)],
                        s_sb[:, ts((n % 2) * NFULL + c, NTILE)],
                        start=False,
                        stop=False,
                    )
                nc.tensor.matmul(
                    pb[:],
                    wr_sb[:],
                    sb_b[:, ts(n, NTILE)],
                    start=False,
                    stop=True,
                )

                if n % 2 == 0:
                    st = stage.tile([NH, 2 * NTILE], dt.float8e4)
                for half in range(2):
                    t_idx = 2 * n + half
                    cur = pb[:, ts(half, B_CORE)]
                    m_src = mems[t_idx % 2]
                    m_dst = mems[(t_idx + 1) % 2]
                    t1 = tpool.tile([NH, B_CORE], dt.float32)
                    # t1 = cur * 2^-11 - prev_spk
                    nc.vector.scalar_tensor_tensor(
                        t1[:], cur, 1.0 / SCALE, prev_spk, Alu.mult, Alu.subtract
                    )
                    # mem' = mem * beta + t1  (step 0: mem_init is a per-row const)
                    if t_idx == 0:
                        nc.vector.tensor_scalar(
                            m_dst[:], t1[:], bc0_sb, None, Alu.add
                        )
                    else:
                        nc.vector.scalar_tensor_tensor(
                            m_dst[:], m_src[:], BETA, t1[:], Alu.mult, Alu.add
                        )
                    # spk = (mem' > thr) in {0,1}, fp8
                    spk_ap = st[:, ts(2 * (n % 2) + half, B_CORE)]
                    nc.vector.tensor_scalar(
                        spk_ap, m_dst[:], thr_sb[:], None, Alu.is_gt
                    )
                    prev_spk = spk_ap

                if n == NPAIR - 2:
                    # ship the penultimate pair early so the final DMA is small
                    nc.scalar.dma_start(
                        spk_out[n // 2, :, 0 : 2 * B_CORE], st[:, 0 : 2 * B_CORE]
                    )
                elif n == NPAIR - 1:
                    nc.scalar.dma_start(
                        spk_out[n // 2, :, 2 * B_CORE :], st[:, 2 * B_CORE :]
                    )
                elif n % 2 == 1:
                    nc.scalar.dma_start(spk_out[n // 2], st[:])
                if n == 1:
                    # deferred: big remainder xfer would starve the spike queue
                    # during the startup crunch if issued up front
                    nc.scalar.dma_start(sb_b[:, 8 * NTILE :], spk_b[:, 8 * NTILE :])

    nc.compile()
    return nc


def _gen_spikes(data_it):
    import jax

    cpu = jax.devices("cpu")[0]
    with jax.default_device(cpu):
        key = jax.random.key(42)
        spikes = jax.random.bernoulli(
            key, jax.numpy.asarray(data_it), (T,) + data_it.shape
        )
        return np.asarray(spikes)  # bool [T, B, NIN]


def _setup_trace_hooks():
    try:
        import antenv.axon_hooks  # noqa: F401

        return
    except ImportError:
        pass
    try:
        import types

        from trn_agent_boot.trn_boot import _ntff_profile_via_ctypes

        hook = _ntff_profile_via_ctypes("/opt/axon/libaxon_pjrt.so")
        if hook is None:
            return
        m = types.ModuleType("antenv.axon_hooks")
        m.get_axon_ntff_profile_hook = lambda: hook
        m.set_axon_ntff_profile_hook = lambda h: None
        sys.modules["antenv.axon_hooks"] = m
        from concourse import bass_utils as bu

        bu.upload_artifacts = lambda tmpdir: "local://skipped"
    except Exception as e:  # pragma: no cover
        print(f"trace hook setup failed: {e}", file=sys.stderr)


def kernel(data_it, w1, b1, w2, b2, num_steps):
    global last_exec_time_ns
    assert int(num_steps) == T

    from concourse import bass_utils

    if "nc" not in _CACHE:
        _CACHE["nc"] = _build_bass()
    nc = _CACHE["nc"]

    data_it = np.asarray(data_it, np.float32)
    w1 = np.asarray(w1, np.float32)
    b1 = np.asarray(b1, np.float32)
    w2 = np.asarray(w2, np.float32)
    b2 = np.asarray(b2, np.float32)

    spikes = _gen_spikes(data_it)  # bool [T, B, NIN]

    # weights: w1T [784, 128] -> hi/lo fp16 split, both pre-scaled by 2^11
    w1t = np.ascontiguousarray(w1.T)  # [784, 128]
    hi = w1t.astype(np.float16)
    lo32 = (w1t - hi.astype(np.float32)) * SCALE
    wh = (hi.astype(np.float32) * SCALE).astype(np.float16)
    wl = lo32.astype(np.float16)
    # full chunks: interleave [hi0, lo0, hi1, lo1, ...] along free dim
    wmain = np.stack(
        [
            wh[: NFULL * 128].reshape(NFULL, 128, NH),
            wl[: NFULL * 128].reshape(NFULL, 128, NH),
        ],
        axis=1,
    ).reshape(2 * NFULL, 128, NH)
    wmain = np.ascontiguousarray(
        wmain.transpose(1, 0, 2).reshape(128, 2 * NFULL * NH)
    )
    wrem = np.ascontiguousarray(
        np.concatenate([wh[NFULL * 128 :], wl[NFULL * 128 :]], axis=0)
    )  # [32, 128]

    c_shift = (b1 / (1.0 - BETA)).astype(np.float32)  # [128]
    init_vec = np.ascontiguousarray(
        np.stack([THR - c_shift, -BETA * c_shift], axis=1), np.float32
    )  # [128, 2]: col0 = threshold, col1 = beta * mem_init

    f8 = ml_dtypes.float8_e4m3
    in_maps = []
    for i in range(N_CORES):
        s = spikes[:, i * B_CORE : (i + 1) * B_CORE, :]  # [100, 256, 784] bool
        s = s.reshape(NPAIR, 2, B_CORE, NIN).astype(np.uint8)
        sa = s[..., : NFULL * 128].transpose(0, 3, 1, 2)  # [50, 768, 2, 256]
        sa = (
            sa.reshape(NPAIR, NFULL, 128, NTILE)
            .transpose(0, 2, 1, 3)
            .reshape(NPAIR // 2, 2, 128, NFULL * NTILE)
            .transpose(0, 2, 1, 3)
            .reshape(NPAIR // 2, 128, 2 * NFULL * NTILE)
        )
        sa = np.ascontiguousarray(sa).astype(f8)
        sb = s[..., NFULL * 128 :].transpose(0, 3, 1, 2).reshape(NPAIR, KREM, NTILE)
        sb = np.concatenate([sb, sb], axis=1)  # [50, 32, 512]
        sb = np.ascontiguousarray(
            sb.transpose(1, 0, 2).reshape(2 * KREM, NPAIR * NTILE)
        ).astype(f8)
        in_maps.append(
            {
                "spk_a": sa,
                "spk_b": sb,
                "wmain": wmain,
                "wrem": wrem,
                "init_vec": init_vec,
            }
        )

    trace = bool(os.environ.get("BASS_TRACE"))
    kw = {}
    if trace:
        _setup_trace_hooks()
        tdir = os.environ.get("BASS_TRACE_DIR")
        if tdir:
            os.makedirs(tdir, exist_ok=True)
            kw["tmpdir"] = tdir
    res = bass_utils.run_bass_kernel_spmd(
        nc, in_maps, list(range(N_CORES)), trace=trace, **kw
    )
    last_exec_time_ns = res.exec_time_ns

    # gather spk1: per core [25, 128, 1024] fp8; free = (pair k, half, batch)
    spk1_cores = []
    for i in range(N_CORES):
        o = np.asarray(res.results[i]["spk_out"]).astype(np.float32)
        o = o.reshape(NPAIR // 2, NH, 4, B_CORE).transpose(0, 2, 3, 1)
        spk1_cores.append(o.reshape(T, B_CORE, NH))
    spk1 = np.concatenate(spk1_cores, axis=1)  # [100, 2048, 128]

    # layer 2 on host, f32 like the reference
    cur2 = spk1.reshape(-1, NH) @ w2.T.astype(np.float32)
    cur2 = (cur2 + b2).reshape(T, B, NOUT).astype(np.float32)
    mem2 = np.zeros((B, NOUT), np.float32)
    spk2 = np.zeros((B, NOUT), np.float32)
    spk2_rec = np.empty((T, B, NOUT), np.float32)
    mem2_rec = np.empty((T, B, NOUT), np.float32)
    for t in range(T):
        mem2 = BETA * mem2 + cur2[t] - spk2 * THR
        spk2 = (mem2 > THR).astype(np.float32)
        spk2_rec[t] = spk2
        mem2_rec[t] = mem2
    return spk2_rec, mem2_rec
